# revision 5
# baseline (speedup 1.0000x reference)
"""Trainium2 Bass kernel for nn_NeuronCircuitDown (moe_routing).

Math (per token t):
  y[t, :]  = sum_n iw[t, n] * (x[t, :] @ Wn[n])          # projection, Wn: [D, R]
  then K=4 sequential Householder reflections with vectors gathered from a
  32-row table by process_indices:
  y <- y - 2 * v * (v . y)   (v rows are unit-norm up to 1e-7, so the
                              reference's re-normalization is a no-op at
                              fp32 precision and is skipped here)

Distribution: data-parallel over the 16384 tokens, 2048 tokens per core on 8
cores; weights/table replicated.

Per-core device pipeline (tokens on partitions, 16 groups of 128):
  1. PE transposes x tile -> xT (contraction dim on partitions)
  2. fp32r matmuls: proj[t, n*128+r] accumulated over 8 K-chunks
  3. ACT: per-n scaled PSUM->SBUF evacuation (scale = per-partition iw column)
  4. DVE: 3-level tree add over n  -> y
  5. Householder: 4 x (fused dot via tensor_tensor_reduce + fused update via
     scalar_tensor_tensor); reflection vectors pre-gathered with indirect DMA.
"""

import sys

if "/opt/trn_rl_repo" not in sys.path:
    sys.path.insert(0, "/opt/trn_rl_repo")

import numpy as np

B, S, D, R, N_IN, N_PROC, K = 4, 4096, 1024, 128, 8, 32, 4
N_CORES = 8
T_TOTAL = B * S
T = T_TOTAL // N_CORES   # tokens per core
G = T // 128             # token groups of 128 per core
KC = D // 128            # contraction chunks

_cache = {}
last_results = None


def _build_program():
    import concourse.bass as bass
    import concourse.mybir as mybir
    import concourse.tile as tile
    from concourse import bacc

    f32 = mybir.dt.float32
    f32r = mybir.dt.float32r
    i32 = mybir.dt.int32
    mult = mybir.AluOpType.mult
    add = mybir.AluOpType.add
    Copy = mybir.ActivationFunctionType.Copy

    nc = bacc.Bacc(
        "TRN2",
        target_bir_lowering=False,
        debug=False,
        enable_asserts=False,
        num_devices=N_CORES,
    )

    x_d = nc.dram_tensor("x", [T, D], f32r, kind="ExternalInput").ap()
    iw_d = nc.dram_tensor("iw", [T, N_IN], f32, kind="ExternalInput").ap()
    idx_d = nc.dram_tensor("pidx", [T, K], i32, kind="ExternalInput").ap()
    wf_d = nc.dram_tensor("wflat", [D, N_IN * R], f32r, kind="ExternalInput").ap()
    id_d = nc.dram_tensor("ident", [128, 128], f32r, kind="ExternalInput").ap()
    pn_d = nc.dram_tensor("pneur", [N_PROC, R], f32, kind="ExternalInput").ap()
    out_d = nc.dram_tensor("out", [T, R], f32, kind="ExternalOutput").ap()

    with tile.TileContext(nc) as tc:
        with (
            tc.tile_pool(name="const", bufs=1) as cpool,
            tc.tile_pool(name="big", bufs=1) as bigpool,
            tc.tile_pool(name="xin", bufs=3) as xpool,
            tc.tile_pool(name="xt", bufs=3) as xtpool,
            tc.tile_pool(name="scl", bufs=3) as sclpool,
            tc.tile_pool(name="psxt", bufs=2, space="PSUM") as psA,
            tc.tile_pool(name="psproj", bufs=2, space="PSUM") as psB,
        ):
            # ---- constants / prefetches ----
            iw_sb = cpool.tile([128, G, N_IN], f32)
            nc.sync.dma_start(iw_sb[:], iw_d.rearrange("(g p) n -> p g n", p=128))
            idx_sb = cpool.tile([128, G, K], i32)
            nc.sync.dma_start(idx_sb[:], idx_d.rearrange("(g p) k -> p g k", p=128))
            idm = cpool.tile([128, 128], f32r)
            nc.sync.dma_start(idm[:], id_d[:])
            wf_sb = cpool.tile([128, KC, N_IN * R], f32r)
            nc.sync.dma_start(wf_sb[:], wf_d.rearrange("(c p) m -> p c m", p=128))

            # Repack indices to [128, K, G] so gather offset APs are contiguous
            idx2 = cpool.tile([128, K, G], i32)
            for k in range(K):
                nc.vector.tensor_copy(idx2[:, k, :], idx_sb[:, :, k])

            # Pre-gather all reflection vectors: v_all[p, k, g, :] = pneur[idx[g*128+p, k]]
            # (multi-index-per-partition indirect DMAs are broken on HW, so one
            #  gather per (k, g) with a [128, 1] offset column)
            v_all = bigpool.tile([128, K, G, R], f32)
            for k in range(K):
                for g in range(G):
                    nc.gpsimd.indirect_dma_start(
                        out=v_all[:, k, g, :],
                        out_offset=None,
                        in_=pn_d[:],
                        in_offset=bass.IndirectOffsetOnAxis(
                            ap=idx2[:, k, g:g + 1], axis=0
                        ),
                    )

            y_all = bigpool.tile([128, G, R], f32)

            # ---- projection + weighted sum, per token group ----
            for g in range(G):
                x_g = xpool.tile([128, D], f32r, tag="x")
                nc.sync.dma_start(x_g[:], x_d[g * 128:(g + 1) * 128, :])

                ps_xt = psA.tile([128, KC, 128], f32r, tag="psxt")
                for c in range(KC):
                    nc.tensor.transpose(
                        ps_xt[:, c, :], x_g[:, c * 128:(c + 1) * 128], idm[:]
                    )
                xt_g = xtpool.tile([128, KC, 128], f32r, tag="xt")
                nc.scalar.copy(xt_g[:], ps_xt[:])

                ps_proj = psB.tile([128, N_IN * R], f32, tag="psproj")
                for c in range(KC):
                    for h in range(2):
                        nc.tensor.matmul(
                            ps_proj[:, h * 512:(h + 1) * 512],
                            lhsT=xt_g[:, c, :],
                            rhs=wf_sb[:, c, h * 512:(h + 1) * 512],
                            start=(c == 0),
                            stop=(c == KC - 1),
                        )

                scl = sclpool.tile([128, N_IN, R], f32, tag="scl")
                for n in range(N_IN):
                    nc.scalar.activation(
                        scl[:, n, :],
                        ps_proj[:, n * R:(n + 1) * R],
                        Copy,
                        bias=0.0,
                        scale=iw_sb[:, g, n:n + 1],
                    )
                t1 = sclpool.tile([128, 4, R], f32, tag="t1")
                nc.vector.tensor_tensor(
                    out=t1[:], in0=scl[:, 0:4, :], in1=scl[:, 4:8, :], op=add
                )
                t2 = sclpool.tile([128, 2, R], f32, tag="t2")
                nc.vector.tensor_tensor(
                    out=t2[:], in0=t1[:, 0:2, :], in1=t1[:, 2:4, :], op=add
                )
                nc.vector.tensor_tensor(
                    out=y_all[:, g, :], in0=t2[:, 0, :], in1=t2[:, 1, :], op=add
                )

            # ---- Householder chain ----
            md2 = bigpool.tile([128, G], f32)
            hh_scr = bigpool.tile([128, G, R], f32)
            for k in range(K):
                for g in range(G):
                    # hh_scr = (-2*y) * v ; md2 = sum(hh_scr) = -2 * (v . y)
                    nc.vector.scalar_tensor_tensor(
                        out=hh_scr[:, g, :],
                        in0=y_all[:, g, :],
                        scalar=-2.0,
                        in1=v_all[:, k, g, :],
                        op0=mult,
                        op1=mult,
                        accum_out=md2[:, g:g + 1],
                    )
                    # y = v * md2 + y
                    nc.vector.scalar_tensor_tensor(
                        out=y_all[:, g, :],
                        in0=v_all[:, k, g, :],
                        scalar=md2[:, g:g + 1],
                        in1=y_all[:, g, :],
                        op0=mult,
                        op1=add,
                    )

            nc.sync.dma_start(out_d.rearrange("(g p) r -> p g r", p=128), y_all[:])

    nc.compile()
    return nc


def _get_program():
    if "nc" not in _cache:
        _cache["nc"] = _build_program()
    return _cache["nc"]


def _host_prep(x, input_weights, process_indices, input_neurons, process_neurons):
    xf = np.ascontiguousarray(np.asarray(x, dtype=np.float32)).reshape(T_TOTAL, D)
    iwf = np.ascontiguousarray(np.asarray(input_weights, dtype=np.float32)).reshape(
        T_TOTAL, N_IN
    )
    idxf = np.ascontiguousarray(np.asarray(process_indices, dtype=np.int32)).reshape(
        T_TOTAL, K
    )
    wflat = np.ascontiguousarray(
        np.transpose(np.asarray(input_neurons, dtype=np.float32), (1, 0, 2)).reshape(
            D, N_IN * R
        )
    )
    ident = np.eye(128, dtype=np.float32)
    pneur = np.ascontiguousarray(np.asarray(process_neurons, dtype=np.float32))
    in_maps = []
    for c in range(N_CORES):
        sl = slice(c * T, (c + 1) * T)
        in_maps.append(
            {
                "x": xf[sl],
                "iw": iwf[sl],
                "pidx": idxf[sl],
                "wflat": wflat,
                "ident": ident,
                "pneur": pneur,
            }
        )
    return in_maps


def kernel(x, input_weights, process_indices, input_neurons, process_neurons):
    global last_results
    from concourse.bass_utils import run_bass_kernel_spmd

    nc = _get_program()
    in_maps = _host_prep(
        x, input_weights, process_indices, input_neurons, process_neurons
    )
    res = run_bass_kernel_spmd(nc, in_maps, core_ids=list(range(N_CORES)))
    last_results = res
    out = np.concatenate([res.results[c]["out"] for c in range(N_CORES)], axis=0)
    return out.reshape(B, S, R)
